# revision 2
# baseline (speedup 1.0000x reference)
"""CPDBlock (rank-decomposed conv block) Trainium2 kernel.

Reference computation (per image):
  y1 = (sum_r w_head[r]) @ x            # 1x1 conv, 256->256
  y2 = conv_(3,1)(y1, w_body)           # 256->64, pad (1,0) in H
  y3 = conv_(1,3)(y2, w_tail) + b_tail  # 64->256, pad (0,1) in W

Algebraic fusion: head folds into body since both are linear:
  y2[r,h,w] = sum_kh (w_body[:, :, kh] @ w_sum) @ x[:, h+kh-1, w]
so the kernel only runs two conv stages:
  fused:  Wc[kh] = w_body[kh] @ w_head.sum(0)  (3x [64,256], host-side)
  tail:   w_tail as-is, bias folded into the PSUM->SBUF copy.

Sharding: data-parallel over batch, 16 images / 8 cores = 2 images/core.
All matmuls run at 1 cycle/row (fp16 fused, f32r tail).

PE stream floor is 10 K-passes per output position (fused 6 + tail 4),
~104.5 us/core; the remaining time is startup + drain + stalls.  This
version software-pipelines the tail one group-pair behind the fused
stage (the PSUM->y2d copies on ACT overlap tail matmuls of the
previous pair instead of stalling PE), loads a small first x piece so
PE starts early, and defers the tail-weight/bias DMAs behind the first
x pieces.  Output DMA is emitted per pair right after its tail (the
final pair per group) to shrink the end-of-kernel drain.

Layout per core, per image, H processed in chunks of HC rows:
  x chunk  [128p=cin%128, 2=cin//128, HC+2 rows (halo), 112]  (SBUF, fp16)
  y2d      [128p, HC rows, 114]: partitions 0-63 hold y2 row-padded
           (col0=0, cols1..112=data), partitions 64-127 hold the same
           shifted one col left (cols0..111=data, col112=0).  This gives
           the tail's three shifted W-windows as plain AP offsets with
           K=128 matmuls (third tap uses a half-zero lhsT).
  y3 stage [128p=cout%128, 2=cout//128, HC, 112] -> DMA out (fp16).
"""
import os

import numpy as np

import concourse.bass as bass
import concourse.mybir as mybir
import concourse.tile as tile
from concourse import bacc
from concourse.bass_utils import run_bass_kernel_spmd

F32 = mybir.dt.float32
F32R = mybir.dt.float32r
F16 = mybir.dt.float16

B, CIN, COUT, RANK, H, W = 16, 256, 256, 64, 112, 112
NCORES = 8
BL = B // NCORES          # images per core
KO = CIN // 128           # input-channel k-tiles
MO = COUT // 128          # output-channel m-tiles
HC = 56                   # rows per chunk
NCH = H // HC             # chunks per image
NR = 4                    # output rows per matmul group (N = NR*112 = 448)
NG = HC // NR             # groups per chunk

LAST_EXEC_NS = None
LAST_IN_MAPS = None


def _build(reps: int = 1, loop_reps: int = 1):
    fp16 = os.environ.get("CPD_FP16", "1") == "1"
    xdt = F16 if fp16 else F32R
    odt = F16 if fp16 else F32
    nc = bacc.Bacc("TRN2", target_bir_lowering=False, debug=False,
                   num_devices=NCORES)
    x_d = nc.dram_tensor("x", [BL, CIN, H, W], xdt, kind="ExternalInput")
    wf_d = nc.dram_tensor("wf", [128, 3, KO, RANK], xdt, kind="ExternalInput")
    wt_d = nc.dram_tensor("wt", [128, MO, 2, 128], F32R, kind="ExternalInput")
    bias_d = nc.dram_tensor("bias", [128, MO], F32, kind="ExternalInput")
    zeros_d = nc.dram_tensor("zeros", [128, HC], F32R, kind="ExternalInput")
    o_d = nc.dram_tensor("o", [BL, COUT, H, W], odt, kind="ExternalOutput")

    with tile.TileContext(nc) as tc:
        with (
            tc.tile_pool(name="wpool", bufs=1) as wpool,
            tc.tile_pool(name="xpool", bufs=2) as xpool,
            tc.tile_pool(name="ypool", bufs=1) as ypool,
            tc.tile_pool(name="opool",
                         bufs=int(os.environ.get("CPD_OPOOL", "2"))) as opool,
            tc.tile_pool(name="psf", bufs=int(os.environ.get("CPD_PSF", "2")),
                         space="PSUM") as psf,
            tc.tile_pool(name="pst", bufs=int(os.environ.get("CPD_PST", "3")),
                         space="PSUM") as pst,
        ):
            wf = wpool.tile([128, 3, KO, RANK], xdt)
            wt = wpool.tile([128, MO, 2, 128], F32R)
            bias = wpool.tile([128, MO], F32)
            # wf is needed by the very first Ldweights -- load it first.
            # wt/bias/zeros aren't needed until the first tail pair (~10 us
            # in), so their DMAs are deferred until after the first x pieces
            # (HWDGE processes descriptors serially; front-loading them would
            # delay the x transfer PE is waiting on).
            nc.sync.dma_start(wf[:], wf_d[:])

            # Two persistent y2d buffers, manually alternated per chunk.
            # Their pad columns (left pad of the top half, right pad of the
            # bottom half) are zeroed once and never written again.
            y2ds = [ypool.tile([128, HC, 114], F32R, tag=f"y2d{i}",
                               name=f"y2d{i}")
                    for i in range(2)]

            import contextlib
            loop_cm = (tc.For_i(0, loop_reps, 1) if loop_reps > 1
                       else contextlib.nullcontext())
            it = 0
            xt_prev = None
            # Pending tail work item, one group-pair behind the fused stage:
            # (y2d, y3t, ov, h0, gp, subs, last_flag)
            pending = None

            def emit_tail_pair(item):
                y2d, y3t, ov, h0, gp, subs, is_last = item
                for sub in subs:
                    g = gp + sub
                    r0 = g * NR
                    pts = [pst.tile([128, NR * W], F32,
                                    tag=f"pt{mo}", name=f"pt{mo}")
                           for mo in range(MO)]
                    for mo in range(MO):
                        for s in range(2):
                            nc.tensor.matmul(
                                pts[mo][:], wt[:, mo, s, :],
                                y2d[:, r0:r0 + NR, s:112 + s],
                                start=(s == 0), stop=(s == 1))
                    for mo in range(MO):
                        nc.vector.tensor_tensor(
                            y3t[:, mo, r0:r0 + NR, :],
                            pts[mo][:],
                            bias[:, mo, None].to_broadcast([128, NR, W]),
                            mybir.AluOpType.add,
                        )
                    if is_last:
                        # final pair: store per group to shrink the drain
                        nc.sync.dma_start(
                            ov[:, :, h0 + r0:h0 + r0 + NR, :],
                            y3t[:, :, r0:r0 + NR, :])
                if not is_last:
                    r0 = gp * NR
                    r1 = (gp + len(subs)) * NR
                    nc.sync.dma_start(ov[:, :, h0 + r0:h0 + r1, :],
                                      y3t[:, :, r0:r1, :])

            with loop_cm:
              for rep in range(reps):
               for b in range(BL):
                xv = x_d.ap()[b].rearrange("(ko p) h w -> p ko h w", p=128)
                ov = o_d.ap()[b].rearrange("(mo p) h w -> p mo h w", p=128)
                for ch in range(NCH):
                    h0 = ch * HC
                    first_chunk = (rep == 0 and b == 0 and ch == 0)
                    xt = xpool.tile([128, KO, HC + 2, W], xdt)
                    # xt slot i holds absolute image row h0 + i - 1;
                    # edge chunks leave the out-of-image slot unwritten and
                    # skip the matmul term that would read it instead.
                    # Rows h0-1, h0 are copied from the previous chunk's tile
                    # (they were already DMA'd once); each image row is DMA'd
                    # from HBM exactly once.  DMAs are split so the first
                    # groups' matmuls start before the whole chunk lands.
                    if ch == 0:
                        lo = 1
                    else:
                        nc.gpsimd.tensor_copy(xt[:, :, 0:2, :],
                                              xt_prev[:, :, HC:HC + 2, :])
                        lo = 2
                    hi = HC + 2 if ch < NCH - 1 else HC + 1
                    xstep = int(os.environ.get("CPD_XSTEP", "28"))
                    if first_chunk:
                        # small first piece so the first matmul pair (needs
                        # slots 0..9) starts as early as possible
                        bounds = [1, 11]
                        while bounds[-1] < hi:
                            bounds.append(min(bounds[-1] + xstep, hi))
                    else:
                        bounds = list(range(lo, hi, xstep)) + [hi]
                    for s0, s1 in zip(bounds[:-1], bounds[1:]):
                        nc.sync.dma_start(
                            xt[:, :, s0:s1, :],
                            xv[:, :, h0 + s0 - 1:h0 + s1 - 1, :])
                    if first_chunk:
                        nc.sync.dma_start(wt[:], wt_d[:])
                        nc.sync.dma_start(bias[:], bias_d[:])
                        for y2d_ in y2ds:
                            nc.sync.dma_start(y2d_[0:64, :, 0],
                                              zeros_d.ap()[0:64, :])
                            nc.sync.dma_start(y2d_[64:128, :, 112],
                                              zeros_d.ap()[64:128, :])
                    xt_prev = xt

                    y2d = y2ds[it % 2]
                    it += 1
                    y3t = opool.tile([128, MO, HC, W], odt)

                    # Fused-stage groups are processed in pairs: group gp
                    # lands in PSUM partitions 0:64 (PE column-group 0/1),
                    # group gp+1 in partitions 64:128 (column-group 2/3).
                    for gp in range(0, NG, 2):
                        subs = ([0, 1] if gp + 1 < NG else [0])
                        pfp = psf.tile([128, NR * W], F32)
                        for ko in range(KO):
                            for kh in (1, 0, 2):
                                for sub in subs:
                                    g = gp + sub
                                    r0 = g * NR
                                    p0 = 64 * sub
                                    out_ap = pfp[p0:p0 + 64, :]
                                    rhs = xt[:, ko, r0 + kh:r0 + kh + NR, :]
                                    if ch == 0 and g == 0 and kh == 0:
                                        # output row 0 has no row above
                                        out_ap = pfp[p0:p0 + 64, W:NR * W]
                                        rhs = xt[:, ko, 1:NR, :]
                                    elif (ch == NCH - 1 and g == NG - 1
                                          and kh == 2):
                                        # last row has no row below
                                        out_ap = pfp[p0:p0 + 64,
                                                     0:(NR - 1) * W]
                                        rhs = xt[:, ko, r0 + 2:r0 + 1 + NR, :]
                                    nc.tensor.matmul(
                                        out_ap,
                                        wf[:, kh, ko, :],
                                        rhs,
                                        start=(ko == 0 and kh == 1),
                                        stop=(ko == KO - 1 and kh == 2),
                                        tile_position=(0, p0),
                                    )
                        # y2 -> both halves of the padded/shifted layout
                        # (both copies on ACT, off PE's critical path thanks
                        # to the one-pair tail delay below)
                        for sub in subs:
                            g = gp + sub
                            r0 = g * NR
                            p0 = 64 * sub
                            pf = pfp[p0:p0 + 64, :]
                            nc.scalar.copy(y2d[0:64, r0:r0 + NR, 1:113], pf)
                            nc.scalar.copy(y2d[64:128, r0:r0 + NR, 0:112], pf)
                        # tail runs one pair behind the fused stage, so the
                        # ACT copies above overlap PE instead of stalling it
                        if pending is not None:
                            emit_tail_pair(pending)
                        is_last = (rep == reps - 1 and b == BL - 1
                                   and ch == NCH - 1 and gp + 2 >= NG)
                        pending = (y2d, y3t, ov, h0, gp, subs, is_last)
                    if ch == NCH - 1 and b == BL - 1 and rep == reps - 1:
                        emit_tail_pair(pending)
                        pending = None
    nc.compile()
    return nc


_NC_CACHE = None


def kernel(x, w_head, w_body, w_tail, b_tail):
    global _NC_CACHE, LAST_EXEC_NS
    x = np.ascontiguousarray(np.asarray(x, dtype=np.float32))
    w_head = np.asarray(w_head, dtype=np.float32)
    w_body = np.asarray(w_body, dtype=np.float32)
    w_tail = np.asarray(w_tail, dtype=np.float32)
    b_tail = np.asarray(b_tail, dtype=np.float32)

    # --- host-side weight prep (tiny) ---
    w_sum = w_head.astype(np.float64).sum(axis=0)          # [COUT, CIN]
    wc = np.einsum("rok,oi->kri", w_body[:, :, :, 0].astype(np.float64),
                   w_sum)                                  # [3, RANK, CIN]
    # wf[p, kh, ko, m] = Wc[kh][m, ko*128+p]
    wf = np.transpose(wc.reshape(3, RANK, KO, 128), (3, 0, 2, 1))
    wf = np.ascontiguousarray(wf.astype(np.float32))

    # wt[p, mo, 0, m]: p<64 -> w_tail[mo*128+m, p, 0, 0]; p>=64 -> tap1
    #   [p, mo, 1, m]: p<64 -> 0;                         p>=64 -> tap2
    wt = np.zeros((128, MO, 2, 128), dtype=np.float32)
    wtl = w_tail[:, :, 0, :].reshape(MO, 128, RANK, 3)     # [mo, m, r, kw]
    wt[0:64, :, 0, :] = np.transpose(wtl[:, :, :, 0], (2, 0, 1))
    wt[64:128, :, 0, :] = np.transpose(wtl[:, :, :, 1], (2, 0, 1))
    wt[64:128, :, 1, :] = np.transpose(wtl[:, :, :, 2], (2, 0, 1))

    bias = np.ascontiguousarray(b_tail.reshape(MO, 128).T)  # [128, mo]

    fp16 = os.environ.get("CPD_FP16", "1") == "1"
    if fp16:
        x = np.ascontiguousarray(x.astype(np.float16))
        wf = np.ascontiguousarray(wf.astype(np.float16))

    if _NC_CACHE is None:
        _NC_CACHE = _build()
    nc = _NC_CACHE

    zeros = np.zeros((128, HC), dtype=np.float32)
    in_maps = [
        {"x": x[c * BL:(c + 1) * BL], "wf": wf, "wt": wt, "bias": bias,
         "zeros": zeros}
        for c in range(NCORES)
    ]
    global LAST_IN_MAPS
    LAST_IN_MAPS = in_maps
    trace = os.environ.get("KBENCH_TRACE", "0") == "1"
    res = run_bass_kernel_spmd(nc, in_maps, core_ids=list(range(NCORES)),
                               trace=trace)
    LAST_EXEC_NS = res.exec_time_ns
    out = np.concatenate([r["o"] for r in res.results], axis=0)
    if out.dtype != np.float32:
        out = out.astype(np.float32)
    return out


# revision 24
# speedup vs baseline: 14.8442x; 14.8442x over previous
"""CPDBlock (rank-decomposed conv block) Trainium2 kernel.

Reference computation (per image):
  y1 = (sum_r w_head[r]) @ x            # 1x1 conv, 256->256
  y2 = conv_(3,1)(y1, w_body)           # 256->64, pad (1,0) in H
  y3 = conv_(1,3)(y2, w_tail) + b_tail  # 64->256, pad (0,1) in W

Algebraic fusion: head folds into body since both are linear:
  y2[r,h,w] = sum_kh (w_body[:, :, kh] @ w_sum) @ x[:, h+kh-1, w]
so the kernel only runs two conv stages:
  fused:  Wc[kh] = w_body[kh] @ w_head.sum(0)  (3x [64,256], host-side)
  tail:   w_tail as-is, bias folded into the PSUM->SBUF copy.

Sharding: data-parallel over batch, 16 images / 8 cores = 2 images/core.
All matmuls run at 1 cycle/row (fp16 fused, f32r tail).

PE stream floor is 10 K-passes per output position (fused 6 + tail 4),
~104.5 us/core; the remaining time is startup + drain + stalls.  This
version software-pipelines the tail one group-pair behind the fused
stage (the PSUM->y2d copies on ACT overlap tail matmuls of the
previous pair instead of stalling PE), loads a small first x piece so
PE starts early, and defers the tail-weight/bias DMAs behind the first
x pieces.  Output DMA is emitted per pair right after its tail (the
final pair per group) to shrink the end-of-kernel drain.

Layout per core, per image, H processed in chunks of HC rows:
  x chunk  [128p=cin%128, 2=cin//128, HC+2 rows (halo), 112]  (SBUF, fp16)
  y2d      [128p, HC rows, 114]: partitions 0-63 hold y2 row-padded
           (col0=0, cols1..112=data), partitions 64-127 hold the same
           shifted one col left (cols0..111=data, col112=0).  This gives
           the tail's three shifted W-windows as plain AP offsets with
           K=128 matmuls (third tap uses a half-zero lhsT).
  y3 stage [128p=cout%128, 2=cout//128, HC, 112] -> DMA out (fp16).
"""
import os

import numpy as np

import concourse.bass as bass
import concourse.mybir as mybir
import concourse.tile as tile
from concourse import bacc
from concourse.bass_utils import run_bass_kernel_spmd

F32 = mybir.dt.float32
F32R = mybir.dt.float32r
F16 = mybir.dt.float16

B, CIN, COUT, RANK, H, W = 16, 256, 256, 64, 112, 112
NCORES = 8
BL = B // NCORES          # images per core
KO = CIN // 128           # input-channel k-tiles
MO = COUT // 128          # output-channel m-tiles
HC = 56                   # rows per chunk
NCH = H // HC             # chunks per image
NR = 4                    # output rows per matmul group (N = NR*112 = 448)
NG = HC // NR             # groups per chunk

LAST_EXEC_NS = None
LAST_IN_MAPS = None


def _build(reps: int = 1, loop_reps: int = 1):
    fp16 = os.environ.get("CPD_FP16", "1") == "1"
    xdt = F16 if fp16 else F32R
    odt = F16 if fp16 else F32
    nc = bacc.Bacc("TRN2", target_bir_lowering=False, debug=False,
                   num_devices=NCORES)
    x_d = nc.dram_tensor("x", [BL, CIN, H, W], xdt, kind="ExternalInput")
    wf_d = nc.dram_tensor("wf", [128, 3, KO, RANK], xdt, kind="ExternalInput")
    wt_d = nc.dram_tensor("wt", [128, MO, 2, 128], F32R, kind="ExternalInput")
    bias_d = nc.dram_tensor("bias", [128, MO], F32, kind="ExternalInput")
    o_d = nc.dram_tensor("o", [BL, COUT, H, W], odt, kind="ExternalOutput")

    with tile.TileContext(nc) as tc:
        with (
            tc.tile_pool(name="wpool", bufs=1) as wpool,
            tc.tile_pool(name="xpool", bufs=2) as xpool,
            tc.tile_pool(name="ypool", bufs=1) as ypool,
            tc.tile_pool(name="opool",
                         bufs=int(os.environ.get("CPD_OPOOL", "2"))) as opool,
            tc.tile_pool(name="psf", bufs=int(os.environ.get("CPD_PSF", "2")),
                         space="PSUM") as psf,
            tc.tile_pool(name="pst", bufs=int(os.environ.get("CPD_PST", "3")),
                         space="PSUM") as pst,
        ):
            wf = wpool.tile([128, 3, KO, RANK], xdt)
            wt = wpool.tile([128, MO, 2, 128], F32R)
            bias = wpool.tile([128, MO], F32)
            # wf is needed by the very first Ldweights -- load it first
            # (tiny).  wt/bias aren't needed until the first tail pair
            # (~10 us in), so their DMAs are deferred until after the first
            # x pieces (HWDGE processes descriptors serially; front-loading
            # them would delay the x transfer PE is waiting on).  A software
            # DGE (gpsimd) load for wf measures slower, and starting PE
            # earlier than the x feed rate sustains just causes
            # starve/re-ramp cycles on the tensor engine.
            nc.sync.dma_start(wf[:], wf_d[:])

            # Two persistent y2d buffers, manually alternated per chunk.
            # Their pad columns (left pad of the top half, right pad of the
            # bottom half) are zeroed once (Pool memset: a DMA'd zero column
            # would be a 4-byte-element scatter, ~1.6 us each on the DMA
            # engines right when PE is starving for x) and never written
            # again.
            y2ds = [ypool.tile([128, HC, 114], F32R, tag=f"y2d{i}",
                               name=f"y2d{i}")
                    for i in range(2)]
            for y2d_ in y2ds:
                # bitcast: Memset's ISA check rejects f32r set-values
                nc.gpsimd.memset(y2d_[0:64, :, 0].bitcast(F32), 0.0)
                nc.gpsimd.memset(y2d_[64:128, :, 112].bitcast(F32), 0.0)

            import contextlib
            loop_cm = (tc.For_i(0, loop_reps, 1) if loop_reps > 1
                       else contextlib.nullcontext())
            it = 0
            xt_prev = None
            # Pending tail work item, one group-pair behind the fused stage:
            # (y2d, y3t, ov, h0, gp, subs, last_flag)
            pending = None

            def emit_tail_pair(item):
                y2d, y3t, ov, h0, gp, subs, is_last = item
                for sub in subs:
                    g = gp + sub
                    r0 = g * NR
                    pts = [pst.tile([128, NR * W], F32,
                                    tag=f"pt{mo}", name=f"pt{mo}")
                           for mo in range(MO)]
                    for mo in range(MO):
                        for s in range(2):
                            nc.tensor.matmul(
                                pts[mo][:], wt[:, mo, s, :],
                                y2d[:, r0:r0 + NR, s:112 + s],
                                start=(s == 0), stop=(s == 1))
                    for mo in range(MO):
                        if is_last and mo == 1:
                            # final unit: mo1's bias-add on ACT, parallel
                            # with mo0's on DVE, to shorten the drain
                            nc.scalar.add(y3t[:, mo, r0:r0 + NR, :],
                                          pts[mo][:], bias[:, mo, None])
                        else:
                            nc.vector.tensor_tensor(
                                y3t[:, mo, r0:r0 + NR, :],
                                pts[mo][:],
                                bias[:, mo, None].to_broadcast([128, NR, W]),
                                mybir.AluOpType.add,
                            )
                    if is_last:
                        nc.sync.dma_start(
                            ov[:, :, h0 + r0:h0 + r0 + NR, :],
                            y3t[:, :, r0:r0 + NR, :])
                if not is_last:
                    r0 = gp * NR
                    r1 = (gp + len(subs)) * NR
                    nc.sync.dma_start(ov[:, :, h0 + r0:h0 + r1, :],
                                      y3t[:, :, r0:r1, :])

            with loop_cm:
              for rep in range(reps):
               for b in range(BL):
                xv = x_d.ap()[b].rearrange("(ko p) h w -> p ko h w", p=128)
                ov = o_d.ap()[b].rearrange("(mo p) h w -> p mo h w", p=128)
                for ch in range(NCH):
                    h0 = ch * HC
                    first_chunk = (rep == 0 and b == 0 and ch == 0)
                    xt = xpool.tile([128, KO, HC + 2, W], xdt)
                    # xt slot i holds absolute image row h0 + i - 1;
                    # edge chunks leave the out-of-image slot unwritten and
                    # skip the matmul term that would read it instead.
                    # Rows h0-1, h0 are copied from the previous chunk's tile
                    # (they were already DMA'd once); each image row is DMA'd
                    # from HBM exactly once.  DMAs are split so the first
                    # groups' matmuls start before the whole chunk lands.
                    if ch == 0:
                        lo = 1
                    else:
                        nc.gpsimd.tensor_copy(xt[:, :, 0:2, :],
                                              xt_prev[:, :, HC:HC + 2, :])
                        lo = 2
                    hi = HC + 2 if ch < NCH - 1 else HC + 1
                    xstep = int(os.environ.get("CPD_XSTEP", "28"))
                    if first_chunk:
                        # graded pieces: pair p's fused matmuls need slots up
                        # to 8p+9, so early pieces are small to start PE fast
                        # and stay ahead of consumption
                        bounds = [1, 11, 19, 31]
                        while bounds[-1] < hi:
                            bounds.append(min(bounds[-1] + xstep, hi))
                        bounds = [b for b in bounds if b <= hi]
                        if bounds[-1] != hi:
                            bounds.append(hi)
                    else:
                        bounds = list(range(lo, hi, xstep)) + [hi]
                    for pi, (s0, s1) in enumerate(zip(bounds[:-1],
                                                      bounds[1:])):
                        nc.sync.dma_start(
                            xt[:, :, s0:s1, :],
                            xv[:, :, h0 + s0 - 1:h0 + s1 - 1, :])
                        if first_chunk and pi == 1:
                            # tail weights are first read ~11 us in; queue
                            # them after the x pieces that PE needs sooner
                            nc.sync.dma_start(wt[:], wt_d[:])
                            nc.sync.dma_start(bias[:], bias_d[:])
                    xt_prev = xt

                    y2d = y2ds[it % 2]
                    it += 1
                    y3t = opool.tile([128, MO, HC, W], odt)

                    # Fused-stage groups are processed in pairs: group gp
                    # lands in PSUM partitions 0:64 (PE column-group 0/1),
                    # group gp+1 in partitions 64:128 (column-group 2/3).
                    # Matmul stream time scales with N only, so single-group
                    # units cost the same PE time as paired ones; the first
                    # two and last two groups of the run go as singles so
                    # their tails wait on 2 ACT copies instead of 4
                    # (warmup / flush stalls).
                    units = [(gp, [0, 1] if gp + 1 < NG else [0])
                             for gp in range(0, NG, 2)]
                    if first_chunk:
                        units = [(0, [0]), (1, [0])] + [
                            (gp, [0, 1]) for gp in range(2, NG, 2)]
                    last_chunk = (rep == reps - 1 and b == BL - 1
                                  and ch == NCH - 1)
                    if last_chunk:
                        units = [(gp, [0, 1]) for gp in range(0, NG - 2, 2)
                                 ] + [(NG - 2, [0]), (NG - 1, [0])]
                    for gp, subs in units:
                        pfp = psf.tile([128, NR * W], F32)
                        for ko in range(KO):
                            for kh in (1, 0, 2):
                                for sub in subs:
                                    g = gp + sub
                                    r0 = g * NR
                                    p0 = 64 * sub
                                    out_ap = pfp[p0:p0 + 64, :]
                                    rhs = xt[:, ko, r0 + kh:r0 + kh + NR, :]
                                    if ch == 0 and g == 0 and kh == 0:
                                        # output row 0 has no row above
                                        out_ap = pfp[p0:p0 + 64, W:NR * W]
                                        rhs = xt[:, ko, 1:NR, :]
                                    elif (ch == NCH - 1 and g == NG - 1
                                          and kh == 2):
                                        # last row has no row below
                                        out_ap = pfp[p0:p0 + 64,
                                                     0:(NR - 1) * W]
                                        rhs = xt[:, ko, r0 + 2:r0 + 1 + NR, :]
                                    nc.tensor.matmul(
                                        out_ap,
                                        wf[:, kh, ko, :],
                                        rhs,
                                        start=(ko == 0 and kh == 1),
                                        stop=(ko == KO - 1 and kh == 2),
                                        tile_position=(0, p0),
                                    )
                        # y2 -> both halves of the padded/shifted layout.
                        # Both copies on ACT: the tile dep-tracker serializes
                        # same-tile writes across engines anyway (coarse WAW
                        # on y2d), and ACT has the fastest copy, so splitting
                        # engines only lengthens the chain.  They're off PE's
                        # critical path thanks to the one-pair tail delay.
                        for sub in subs:
                            g = gp + sub
                            r0 = g * NR
                            p0 = 64 * sub
                            pf = pfp[p0:p0 + 64, :]
                            nc.scalar.copy(y2d[0:64, r0:r0 + NR, 1:113], pf)
                            nc.scalar.copy(y2d[64:128, r0:r0 + NR, 0:112], pf)
                        # tail runs one pair behind the fused stage, so the
                        # ACT copies above overlap PE instead of stalling it
                        if pending is not None:
                            emit_tail_pair(pending)
                        is_last = (rep == reps - 1 and b == BL - 1
                                   and ch == NCH - 1 and gp + 2 >= NG)
                        pending = (y2d, y3t, ov, h0, gp, subs, is_last)
                    if ch == NCH - 1 and b == BL - 1 and rep == reps - 1:
                        emit_tail_pair(pending)
                        pending = None
    nc.compile()
    return nc


_NC_CACHE = None


def kernel(x, w_head, w_body, w_tail, b_tail):
    global _NC_CACHE, LAST_EXEC_NS
    x = np.ascontiguousarray(np.asarray(x, dtype=np.float32))
    w_head = np.asarray(w_head, dtype=np.float32)
    w_body = np.asarray(w_body, dtype=np.float32)
    w_tail = np.asarray(w_tail, dtype=np.float32)
    b_tail = np.asarray(b_tail, dtype=np.float32)

    # --- host-side weight prep (tiny) ---
    w_sum = w_head.astype(np.float64).sum(axis=0)          # [COUT, CIN]
    wc = np.einsum("rok,oi->kri", w_body[:, :, :, 0].astype(np.float64),
                   w_sum)                                  # [3, RANK, CIN]
    # wf[p, kh, ko, m] = Wc[kh][m, ko*128+p]
    wf = np.transpose(wc.reshape(3, RANK, KO, 128), (3, 0, 2, 1))
    wf = np.ascontiguousarray(wf.astype(np.float32))

    # wt[p, mo, 0, m]: p<64 -> w_tail[mo*128+m, p, 0, 0]; p>=64 -> tap1
    #   [p, mo, 1, m]: p<64 -> 0;                         p>=64 -> tap2
    wt = np.zeros((128, MO, 2, 128), dtype=np.float32)
    wtl = w_tail[:, :, 0, :].reshape(MO, 128, RANK, 3)     # [mo, m, r, kw]
    wt[0:64, :, 0, :] = np.transpose(wtl[:, :, :, 0], (2, 0, 1))
    wt[64:128, :, 0, :] = np.transpose(wtl[:, :, :, 1], (2, 0, 1))
    wt[64:128, :, 1, :] = np.transpose(wtl[:, :, :, 2], (2, 0, 1))

    bias = np.ascontiguousarray(b_tail.reshape(MO, 128).T)  # [128, mo]

    fp16 = os.environ.get("CPD_FP16", "1") == "1"
    if fp16:
        x = np.ascontiguousarray(x.astype(np.float16))
        wf = np.ascontiguousarray(wf.astype(np.float16))

    if _NC_CACHE is None:
        _NC_CACHE = _build()
    nc = _NC_CACHE

    in_maps = [
        {"x": x[c * BL:(c + 1) * BL], "wf": wf, "wt": wt, "bias": bias}
        for c in range(NCORES)
    ]
    global LAST_IN_MAPS
    LAST_IN_MAPS = in_maps
    trace = os.environ.get("KBENCH_TRACE", "0") == "1"
    res = run_bass_kernel_spmd(nc, in_maps, core_ids=list(range(NCORES)),
                               trace=trace)
    LAST_EXEC_NS = res.exec_time_ns
    out = np.concatenate([r["o"] for r in res.results], axis=0)
    if out.dtype != np.float32:
        out = out.astype(np.float32)
    return out


# revision 28
# speedup vs baseline: 23.1800x; 1.5616x over previous
"""CPDBlock (rank-decomposed conv block) Trainium2 kernel.

Reference computation (per image):
  y1 = (sum_r w_head[r]) @ x            # 1x1 conv, 256->256
  y2 = conv_(3,1)(y1, w_body)           # 256->64, pad (1,0) in H
  y3 = conv_(1,3)(y2, w_tail) + b_tail  # 64->256, pad (0,1) in W

Algebraic fusion: head folds into body since both are linear:
  y2[r,h,w] = sum_kh (w_body[:, :, kh] @ w_sum) @ x[:, h+kh-1, w]
so the kernel only runs two conv stages:
  fused:  Wc[kh] = w_body[kh] @ w_head.sum(0)  (3x [64,256], host-side)
  tail:   w_tail as-is, bias folded into the PSUM->SBUF copy.

Sharding: data-parallel over batch, 16 images / 8 cores = 2 images/core.
All matmuls run at 1 cycle/row (fp16 fused, f32r tail).

PE stream floor is 10 K-passes per output position (fused 6 + tail 4),
~104.5 us/core; the remaining time is startup + drain + stalls.  This
version software-pipelines the tail one group-pair behind the fused
stage (the PSUM->y2d copies on ACT overlap tail matmuls of the
previous pair instead of stalling PE), loads a small first x piece so
PE starts early, and defers the tail-weight/bias DMAs behind the first
x pieces.  Output DMA is emitted per pair right after its tail (the
final pair per group) to shrink the end-of-kernel drain.

Layout per core, per image, H processed in chunks of HC rows:
  x chunk  [128p=cin%128, 2=cin//128, HC+2 rows (halo), 112]  (SBUF, fp16)
  y2d      [128p, HC rows, 114]: partitions 0-63 hold y2 row-padded
           (col0=0, cols1..112=data), partitions 64-127 hold the same
           shifted one col left (cols0..111=data, col112=0).  This gives
           the tail's three shifted W-windows as plain AP offsets with
           K=128 matmuls (third tap uses a half-zero lhsT).
  y3 stage [128p=cout%128, 2=cout//128, HC, 112] -> DMA out (fp16).
"""
import os

import numpy as np

import concourse.bass as bass
import concourse.mybir as mybir
import concourse.tile as tile
from concourse import bacc
from concourse.bass_utils import run_bass_kernel_spmd

F32 = mybir.dt.float32
F32R = mybir.dt.float32r
F16 = mybir.dt.float16

B, CIN, COUT, RANK, H, W = 16, 256, 256, 64, 112, 112
NCORES = 8
BL = B // NCORES          # images per core
KO = CIN // 128           # input-channel k-tiles
MO = COUT // 128          # output-channel m-tiles
HC = 56                   # rows per chunk
NCH = H // HC             # chunks per image
NR = 4                    # output rows per matmul group (N = NR*112 = 448)
NG = HC // NR             # groups per chunk

LAST_EXEC_NS = None
LAST_IN_MAPS = None


def _build(reps: int = 1, loop_reps: int = 1, steady: bool = False):
    # steady=True: timing-only variant for hardware-loop throughput
    # measurement -- hoists the one-time setup (wt/bias DMA) out of the
    # loop body and uses uniform x piece sizes, so a loop iteration
    # matches the kernel's steady-state schedule.
    fp16 = os.environ.get("CPD_FP16", "1") == "1"
    xdt = F16 if fp16 else F32R
    odt = F16 if fp16 else F32
    nc = bacc.Bacc("TRN2", target_bir_lowering=False, debug=False,
                   num_devices=NCORES)
    x_d = nc.dram_tensor("x", [BL, CIN, H, W], xdt, kind="ExternalInput")
    wf_d = nc.dram_tensor("wf", [128, 3, KO, RANK], xdt, kind="ExternalInput")
    wt_d = nc.dram_tensor("wt", [128, MO, 2, 128], F32R, kind="ExternalInput")
    bias_d = nc.dram_tensor("bias", [128, MO], F32, kind="ExternalInput")
    o_d = nc.dram_tensor("o", [BL, COUT, H, W], odt, kind="ExternalOutput")

    with tile.TileContext(nc) as tc:
        with (
            tc.tile_pool(name="wpool", bufs=1) as wpool,
            tc.tile_pool(name="xpool", bufs=2) as xpool,
            tc.tile_pool(name="ypool", bufs=1) as ypool,
            tc.tile_pool(name="opool",
                         bufs=int(os.environ.get("CPD_OPOOL", "2"))) as opool,
            # PSUM banks: psf 3 + pst 2x2 = 7 of 8 (each 448-f32 tile is one
            # 2KB bank).  Deeper psf lets the fused stage run further ahead
            # of the PSUM->y2d copies; pst=2 suffices, the tail is consumed
            # promptly.
            tc.tile_pool(name="psf", bufs=int(os.environ.get("CPD_PSF", "3")),
                         space="PSUM") as psf,
            tc.tile_pool(name="pst", bufs=int(os.environ.get("CPD_PST", "2")),
                         space="PSUM") as pst,
        ):
            wf = wpool.tile([128, 3, KO, RANK], xdt)
            wt = wpool.tile([128, MO, 2, 128], F32R)
            bias = wpool.tile([128, MO], F32)
            # wf is needed by the very first Ldweights -- load it first
            # (tiny).  wt/bias aren't needed until the first tail pair
            # (~10 us in), so their DMAs are deferred until after the first
            # x pieces (HWDGE processes descriptors serially; front-loading
            # them would delay the x transfer PE is waiting on).  A software
            # DGE (gpsimd) load for wf measures slower, and starting PE
            # earlier than the x feed rate sustains just causes
            # starve/re-ramp cycles on the tensor engine.
            nc.sync.dma_start(wf[:], wf_d[:])
            if steady:
                nc.sync.dma_start(wt[:], wt_d[:])
                nc.sync.dma_start(bias[:], bias_d[:])

            # Two persistent y2d buffers, manually alternated per chunk.
            # Their pad columns (left pad of the top half, right pad of the
            # bottom half) are zeroed once (Pool memset: a DMA'd zero column
            # would be a 4-byte-element scatter, ~1.6 us each on the DMA
            # engines right when PE is starving for x) and never written
            # again.
            y2ds = [ypool.tile([128, HC, 114], F32R, tag=f"y2d{i}",
                               name=f"y2d{i}")
                    for i in range(2)]
            for y2d_ in y2ds:
                # bitcast: Memset's ISA check rejects f32r set-values
                nc.gpsimd.memset(y2d_[0:64, :, 0].bitcast(F32), 0.0)
                nc.gpsimd.memset(y2d_[64:128, :, 112].bitcast(F32), 0.0)

            import contextlib
            loop_cm = (tc.For_i(0, loop_reps, 1) if loop_reps > 1
                       else contextlib.nullcontext())
            it = 0
            xt_prev = None
            # Pending tail work item, one group-pair behind the fused stage:
            # (y2d, y3t, ov, h0, gp, subs, last_flag)
            pending = None

            def emit_tail_pair(item):
                y2d, y3t, ov, h0, gp, subs, is_last = item
                for sub in subs:
                    g = gp + sub
                    r0 = g * NR
                    pts = [pst.tile([128, NR * W], F32,
                                    tag=f"pt{mo}", name=f"pt{mo}")
                           for mo in range(MO)]
                    for mo in range(MO):
                        for s in range(2):
                            nc.tensor.matmul(
                                pts[mo][:], wt[:, mo, s, :],
                                y2d[:, r0:r0 + NR, s:112 + s],
                                start=(s == 0), stop=(s == 1))
                    for mo in range(MO):
                        if is_last and mo == 1:
                            # final unit: mo1's bias-add on ACT, parallel
                            # with mo0's on DVE, to shorten the drain
                            nc.scalar.add(y3t[:, mo, r0:r0 + NR, :],
                                          pts[mo][:], bias[:, mo, None])
                        else:
                            nc.vector.tensor_tensor(
                                y3t[:, mo, r0:r0 + NR, :],
                                pts[mo][:],
                                bias[:, mo, None].to_broadcast([128, NR, W]),
                                mybir.AluOpType.add,
                            )
                    if is_last:
                        nc.sync.dma_start(
                            ov[:, :, h0 + r0:h0 + r0 + NR, :],
                            y3t[:, :, r0:r0 + NR, :])
                if not is_last:
                    r0 = gp * NR
                    r1 = (gp + len(subs)) * NR
                    nc.sync.dma_start(ov[:, :, h0 + r0:h0 + r1, :],
                                      y3t[:, :, r0:r1, :])

            with loop_cm:
              for rep in range(reps):
               for b in range(BL):
                xv = x_d.ap()[b].rearrange("(ko p) h w -> p ko h w", p=128)
                ov = o_d.ap()[b].rearrange("(mo p) h w -> p mo h w", p=128)
                for ch in range(NCH):
                    h0 = ch * HC
                    first_chunk = (rep == 0 and b == 0 and ch == 0
                                   and not steady)
                    xt = xpool.tile([128, KO, HC + 2, W], xdt)
                    # xt slot i holds absolute image row h0 + i - 1;
                    # edge chunks leave the out-of-image slot unwritten and
                    # skip the matmul term that would read it instead.
                    # Rows h0-1, h0 are copied from the previous chunk's tile
                    # (they were already DMA'd once); each image row is DMA'd
                    # from HBM exactly once.  DMAs are split so the first
                    # groups' matmuls start before the whole chunk lands.
                    if ch == 0:
                        lo = 1
                    else:
                        nc.gpsimd.tensor_copy(xt[:, :, 0:2, :],
                                              xt_prev[:, :, HC:HC + 2, :])
                        lo = 2
                    hi = HC + 2 if ch < NCH - 1 else HC + 1
                    xstep = int(os.environ.get("CPD_XSTEP", "28"))
                    if first_chunk:
                        # graded pieces: pair p's fused matmuls need slots up
                        # to 8p+9, so early pieces are small to start PE fast
                        # and stay ahead of consumption
                        bounds = [1, 11, 19, 31]
                        while bounds[-1] < hi:
                            bounds.append(min(bounds[-1] + xstep, hi))
                        bounds = [b for b in bounds if b <= hi]
                        if bounds[-1] != hi:
                            bounds.append(hi)
                    else:
                        bounds = list(range(lo, hi, xstep)) + [hi]
                    for pi, (s0, s1) in enumerate(zip(bounds[:-1],
                                                      bounds[1:])):
                        nc.sync.dma_start(
                            xt[:, :, s0:s1, :],
                            xv[:, :, h0 + s0 - 1:h0 + s1 - 1, :])
                        if first_chunk and pi == 1:
                            # tail weights are first read ~11 us in; queue
                            # them after the x pieces that PE needs sooner
                            nc.sync.dma_start(wt[:], wt_d[:])
                            nc.sync.dma_start(bias[:], bias_d[:])
                    xt_prev = xt

                    y2d = y2ds[it % 2]
                    it += 1
                    y3t = opool.tile([128, MO, HC, W], odt)

                    # Fused-stage groups are processed in pairs: group gp
                    # lands in PSUM partitions 0:64 (PE column-group 0/1),
                    # group gp+1 in partitions 64:128 (column-group 2/3).
                    # Matmul stream time scales with N only, so single-group
                    # units cost the same PE time as paired ones; the first
                    # two and last two groups of the run go as singles so
                    # their tails wait on 2 ACT copies instead of 4
                    # (warmup / flush stalls).
                    units = [(gp, [0, 1] if gp + 1 < NG else [0])
                             for gp in range(0, NG, 2)]
                    if first_chunk:
                        units = [(0, [0]), (1, [0])] + [
                            (gp, [0, 1]) for gp in range(2, NG, 2)]
                    last_chunk = (rep == reps - 1 and b == BL - 1
                                  and ch == NCH - 1)
                    if last_chunk:
                        units = [(gp, [0, 1]) for gp in range(0, NG - 2, 2)
                                 ] + [(NG - 2, [0]), (NG - 1, [0])]
                    for gp, subs in units:
                        pfp = psf.tile([128, NR * W], F32)
                        for ko in range(KO):
                            for kh in (1, 0, 2):
                                for sub in subs:
                                    g = gp + sub
                                    r0 = g * NR
                                    p0 = 64 * sub
                                    out_ap = pfp[p0:p0 + 64, :]
                                    rhs = xt[:, ko, r0 + kh:r0 + kh + NR, :]
                                    if ch == 0 and g == 0 and kh == 0:
                                        # output row 0 has no row above
                                        out_ap = pfp[p0:p0 + 64, W:NR * W]
                                        rhs = xt[:, ko, 1:NR, :]
                                    elif (ch == NCH - 1 and g == NG - 1
                                          and kh == 2):
                                        # last row has no row below
                                        out_ap = pfp[p0:p0 + 64,
                                                     0:(NR - 1) * W]
                                        rhs = xt[:, ko, r0 + 2:r0 + 1 + NR, :]
                                    nc.tensor.matmul(
                                        out_ap,
                                        wf[:, kh, ko, :],
                                        rhs,
                                        start=(ko == 0 and kh == 1),
                                        stop=(ko == KO - 1 and kh == 2),
                                        tile_position=(0, p0),
                                    )
                        # y2 -> both halves of the padded/shifted layout.
                        # Both copies on ACT: the tile dep-tracker serializes
                        # same-tile writes across engines anyway (coarse WAW
                        # on y2d), and ACT has the fastest copy, so splitting
                        # engines only lengthens the chain.  They're off PE's
                        # critical path thanks to the one-pair tail delay.
                        for sub in subs:
                            g = gp + sub
                            r0 = g * NR
                            p0 = 64 * sub
                            pf = pfp[p0:p0 + 64, :]
                            nc.scalar.copy(y2d[0:64, r0:r0 + NR, 1:113], pf)
                            nc.scalar.copy(y2d[64:128, r0:r0 + NR, 0:112], pf)
                        # tail runs one pair behind the fused stage, so the
                        # ACT copies above overlap PE instead of stalling it
                        if pending is not None:
                            emit_tail_pair(pending)
                        is_last = (rep == reps - 1 and b == BL - 1
                                   and ch == NCH - 1 and gp + 2 >= NG)
                        pending = (y2d, y3t, ov, h0, gp, subs, is_last)
                    if ch == NCH - 1 and b == BL - 1 and rep == reps - 1:
                        emit_tail_pair(pending)
                        pending = None
    nc.compile()
    return nc


_NC_CACHE = None


def kernel(x, w_head, w_body, w_tail, b_tail):
    global _NC_CACHE, LAST_EXEC_NS
    x = np.ascontiguousarray(np.asarray(x, dtype=np.float32))
    w_head = np.asarray(w_head, dtype=np.float32)
    w_body = np.asarray(w_body, dtype=np.float32)
    w_tail = np.asarray(w_tail, dtype=np.float32)
    b_tail = np.asarray(b_tail, dtype=np.float32)

    # --- host-side weight prep (tiny) ---
    w_sum = w_head.astype(np.float64).sum(axis=0)          # [COUT, CIN]
    wc = np.einsum("rok,oi->kri", w_body[:, :, :, 0].astype(np.float64),
                   w_sum)                                  # [3, RANK, CIN]
    # wf[p, kh, ko, m] = Wc[kh][m, ko*128+p]
    wf = np.transpose(wc.reshape(3, RANK, KO, 128), (3, 0, 2, 1))
    wf = np.ascontiguousarray(wf.astype(np.float32))

    # wt[p, mo, 0, m]: p<64 -> w_tail[mo*128+m, p, 0, 0]; p>=64 -> tap1
    #   [p, mo, 1, m]: p<64 -> 0;                         p>=64 -> tap2
    wt = np.zeros((128, MO, 2, 128), dtype=np.float32)
    wtl = w_tail[:, :, 0, :].reshape(MO, 128, RANK, 3)     # [mo, m, r, kw]
    wt[0:64, :, 0, :] = np.transpose(wtl[:, :, :, 0], (2, 0, 1))
    wt[64:128, :, 0, :] = np.transpose(wtl[:, :, :, 1], (2, 0, 1))
    wt[64:128, :, 1, :] = np.transpose(wtl[:, :, :, 2], (2, 0, 1))

    bias = np.ascontiguousarray(b_tail.reshape(MO, 128).T)  # [128, mo]

    fp16 = os.environ.get("CPD_FP16", "1") == "1"
    if fp16:
        x = np.ascontiguousarray(x.astype(np.float16))
        wf = np.ascontiguousarray(wf.astype(np.float16))

    if _NC_CACHE is None:
        _NC_CACHE = _build()
    nc = _NC_CACHE

    in_maps = [
        {"x": x[c * BL:(c + 1) * BL], "wf": wf, "wt": wt, "bias": bias}
        for c in range(NCORES)
    ]
    global LAST_IN_MAPS
    LAST_IN_MAPS = in_maps
    trace = os.environ.get("KBENCH_TRACE", "0") == "1"
    res = run_bass_kernel_spmd(nc, in_maps, core_ids=list(range(NCORES)),
                               trace=trace)
    LAST_EXEC_NS = res.exec_time_ns
    out = np.concatenate([r["o"] for r in res.results], axis=0)
    if out.dtype != np.float32:
        out = out.astype(np.float32)
    return out
